# revision 31
# baseline (speedup 1.0000x reference)
"""VQ codebook-lookup kernel for Trainium2 (8 NeuronCores, data-parallel over batch).

Computes, for z_e (32,256,32,32) and codebook (1024,256):
  dist[n,k] = fl(fl(||z_n||^2 - 2 z_n.c_k) + ||c_k||^2)   (fp32, reference rounding order)
  ids = argmin_k dist (first-index tie-break), e_k = codebook[ids], e_k_st == e_k.

Internally the negated distance is used (bit-exact under fp32 sign symmetry) so the
row extreme can be computed with GPSIMD max-pool and located with DVE max_index.

Sharding: batch dim 32 split 4 per core across 8 cores; codebook replicated.
"""

from contextlib import ExitStack

import numpy as np

import concourse.bacc as bacc
import concourse.bass as bass
import concourse.mybir as mybir
import concourse.tile as tile
from concourse.bass_utils import run_bass_kernel_spmd

F32 = mybir.dt.float32
I32 = mybir.dt.int32
I16 = mybir.dt.int16
U32 = mybir.dt.uint32

B, C, H, W = 32, 256, 32, 32
HW = H * W              # 1024
K = 1024                # codebook size
NCORES = 8
BPC = B // NCORES       # batches per core = 4
NMT = HW // 128         # m-tiles per batch = 8
KT = K // 128           # codebook partition tiles = 8


def _emit(ctx: ExitStack, tc: "tile.TileContext", z_ap, cb_ap, ek_ap, st_ap, ids_ap, idsc_ap):
    nc = tc.nc

    const = ctx.enter_context(tc.tile_pool(name="const", bufs=1))
    setup = ctx.enter_context(tc.tile_pool(name="setup", bufs=1))
    zpool = ctx.enter_context(tc.tile_pool(name="z", bufs=BPC))
    t1pool = ctx.enter_context(tc.tile_pool(name="t1", bufs=5))
    dpool = ctx.enter_context(tc.tile_pool(name="dist", bufs=6))
    ekpool = ctx.enter_context(tc.tile_pool(name="ek", bufs=3))
    small = ctx.enter_context(tc.tile_pool(name="small", bufs=4))
    zzp = ctx.enter_context(tc.tile_pool(name="zzp", bufs=10))
    scratch = ctx.enter_context(tc.tile_pool(name="scratch", bufs=2))

    psum_mm = ctx.enter_context(tc.tile_pool(name="psmm", bufs=3, space="PSUM"))
    psum_g = ctx.enter_context(tc.tile_pool(name="psg", bufs=1, space="PSUM"))
    psum_tp = ctx.enter_context(tc.tile_pool(name="pstp", bufs=1, space="PSUM"))

    # ---------------- one-time setup ----------------
    # identity matrix for PE transposes: iota(j - p) == 0
    ident_i = const.tile([128, 128], I32)
    nc.gpsimd.iota(ident_i[:], pattern=[[1, 128]], base=0, channel_multiplier=-1)
    ident = const.tile([128, 128], F32)
    nc.vector.tensor_scalar(
        out=ident[:], in0=ident_i[:], scalar1=0, scalar2=None,
        op0=mybir.AluOpType.is_equal,
    )

    ones8 = const.tile([128, 8], F32)
    nc.vector.memset(ones8[:], 1.0)
    ident8_i = setup.tile([128, NMT * 128], I32)
    nc.gpsimd.iota(ident8_i[:], pattern=[[0, NMT], [1, 128]], base=0,
                   channel_multiplier=-1)
    ident8 = const.tile([128, NMT * 128], F32)
    nc.vector.tensor_scalar(
        out=ident8[:], in0=ident8_i[:], scalar1=0, scalar2=None,
        op0=mybir.AluOpType.is_equal,
    )
    ones_row = const.tile([1, 128], F32)
    nc.vector.memset(ones_row[:], 1.0)

    # load codebook (1024,256) as 8 tiles [128k, 256c]; build:
    #   cbT_m2[ch]    [128c, 1024k] = -2 * codebook^T chunk   (matmul rhs)
    #   cbT_plain[ch] [128c, 1024k] = codebook^T chunk         (gather source)
    #   cc_cols [128, 8]  cc[k] = sum_c cb[k,c]^2 (per k-tile column)
    cbT_m2 = [const.tile([128, K], F32, name=f"cbtm2_{ch}", tag=f"cbtm2_{ch}") for ch in range(2)]
    cbT_pl = [const.tile([128, K], F32, name=f"cbtpl_{ch}", tag=f"cbtpl_{ch}") for ch in range(2)]
    cc_cols = const.tile([128, KT], F32)
    cb_all = setup.tile([128, KT * C], F32, tag="cb")
    nc.scalar.dma_start(cb_all[:].rearrange("p (t c) -> p t c", t=KT),
                        cb_ap.rearrange("(t p) c -> p t c", p=128))
    for kt in range(KT):
        cb_t = cb_all[:, kt * C:(kt + 1) * C]
        sq = scratch.tile([128, C], F32, tag="sq")
        nc.scalar.square(sq[:], cb_t)
        nc.vector.reduce_sum(cc_cols[:, kt:kt + 1], sq[:], axis=mybir.AxisListType.X)
        for ch in range(2):
            ps = psum_tp.tile([128, 128], F32, tag="tp")
            nc.tensor.transpose(ps[:], cb_t[:, ch * 128:(ch + 1) * 128], ident[:])
            nc.scalar.mul(cbT_m2[ch][:, kt * 128:(kt + 1) * 128], ps[:], -2.0)
            nc.scalar.copy(cbT_pl[ch][:, kt * 128:(kt + 1) * 128], ps[:])

    # ccn_bcast [128, 1024] : -cc replicated on every partition
    ps = psum_tp.tile([128, 128], F32, tag="tp")
    nc.tensor.transpose(ps[:8, :], cc_cols[:], ident[:])
    ccn_rows8 = const.tile([8, 128], F32)
    nc.scalar.mul(ccn_rows8[:], ps[:8, :], -1.0)
    ccn_row = const.tile([1, K], F32)
    nc.sync.dma_start(ccn_row[:], ccn_rows8[:])
    ccn_bcast = const.tile([128, K], F32)
    for nh in range(2):
        psb = psum_tp.tile([128, 512], F32, tag="tp")
        nc.tensor.matmul(psb[:], ones_row[:], ccn_row[:, nh * 512:(nh + 1) * 512],
                         start=True, stop=True)
        nc.scalar.copy(ccn_bcast[:, nh * 512:(nh + 1) * 512], psb[:])

    # ------------- main: per batch distances/argmin; gathers in 2 half-phases --
    ids_all_i32 = const.tile([8, BPC * 128], I32)
    ids_all_i16 = const.tile([8, BPC * 128], I16)
    idxw_all = const.tile([128, BPC * (HW // 16)], I16)
    idsc_flat = idsc_ap.rearrange("b n -> (b n)")
    z_tiles = []

    def emit_batch(b):
        z_sb = [zpool.tile([128, HW], F32, name=f"z{ch}", tag=f"z{ch}") for ch in range(2)]
        for ch in range(2):
            nc.sync.dma_start(z_sb[ch][:], z_ap[b, ch * 128:(ch + 1) * 128, :])
        z_tiles.append(z_sb)

        # zz for the 8 m-tiles in two half-strips: gram matmuls into a psum
        # strip, then one masked-diagonal multiply + grouped negated row-sum
        zzn8 = zzp.tile([128, NMT], F32, tag="zzn")
        for h4 in range(2):
            psg = psum_g.tile([128, 4 * 128], F32, tag="g")
            for j in range(4):
                mt = 4 * h4 + j
                ms = slice(mt * 128, (mt + 1) * 128)
                ps = psg[:, j * 128:(j + 1) * 128]
                nc.tensor.matmul(ps, z_sb[0][:, ms], z_sb[0][:, ms], start=True, stop=False)
                nc.tensor.matmul(ps, z_sb[1][:, ms], z_sb[1][:, ms], start=False, stop=True)
            gs = scratch.tile([128, 4 * 128], F32, tag="gs")
            nc.vector.tensor_tensor(gs[:], psg[:], ident8[:, 0:512], op=mybir.AluOpType.mult)
            nc.vector.tensor_reduce(
                zzn8[:, 4 * h4:4 * (h4 + 1)], gs[:].rearrange("p (m e) -> p m e", m=4),
                axis=mybir.AxisListType.X, op=mybir.AluOpType.add, negate=True,
            )

        ids_f = small.tile([128, NMT], F32, tag="idsf")
        idx8_all = small.tile([128, NMT * 8], U32, tag="idx8")
        for mt in range(NMT):
            ms = slice(mt * 128, (mt + 1) * 128)
            # psum = -2 * z^T cb^T  (contract C in 2 chunks of 128)
            pmm = psum_mm.tile([128, K], F32, tag="mm")
            for ch in range(2):
                for nh in range(2):
                    nc.tensor.matmul(
                        pmm[:, nh * 512:(nh + 1) * 512],
                        z_sb[ch][:, ms],
                        cbT_m2[ch][:, nh * 512:(nh + 1) * 512],
                        start=(ch == 0), stop=(ch == 1),
                    )
            # t1n = -fl(zz - 2mm)   (scale -1 + negated bias; bit-exact negation)
            t1n = t1pool.tile([128, K], F32, tag="t1")
            nc.scalar.activation(
                t1n[:], pmm[:], mybir.ActivationFunctionType.Identity,
                bias=zzn8[:, mt:mt + 1], scale=-1.0,
            )
            # dneg = -dist = fl(t1n + (-cc))
            dneg = dpool.tile([128, K], F32, tag="dist")
            nc.gpsimd.tensor_tensor(dneg[:], t1n[:], ccn_bcast[:],
                                    op=mybir.AluOpType.add)
            # top-8 of dneg, then first index of the max (jnp.argmin tie-break)
            mx8 = small.tile([128, 8], F32, tag="mx8")
            nc.vector.max(mx8[:], dneg[:])
            nc.vector.max_index(idx8_all[:, mt * 8:(mt + 1) * 8], mx8[:], dneg[:])
        nc.vector.tensor_copy(
            ids_f[:], idx8_all[:].rearrange("p (m e) -> p m e", m=NMT)[:, :, 0:1])

        # transpose ids [128, 8] -> [8, 128] so row t = ids[128t .. 128t+128)
        pst = psum_tp.tile([128, 128], F32, tag="tp")
        nc.tensor.transpose(pst[:8, :], ids_f[:], ident[:])
        nc.vector.tensor_copy(ids_all_i32[:, 128 * b:128 * (b + 1)], pst[:8, :])
        nc.vector.tensor_copy(ids_all_i16[:, 128 * b:128 * (b + 1)],
                              ids_all_i32[:, 128 * b:128 * (b + 1)])

    def emit_gather(b):
        # ids out + DRAM bounce to rewrap: idxs[16c + r, 64 b + s] = ids[b, 16 s + r]
        sl = slice(128 * b, 128 * (b + 1))
        nc.sync.dma_start(ids_ap[b], ids_all_i32[:, sl])
        nc.sync.dma_start(idsc_ap[b], ids_all_i16[:, sl])
        idw16 = small.tile([16, 64], I16, tag="idw16")
        nc.sync.dma_start(
            idw16[:],
            idsc_flat[1024 * b:1024 * (b + 1)].rearrange("(s r) -> r s", r=16))
        for c8 in range(8):
            nc.sync.dma_start(idxw_all[16 * c8:16 * (c8 + 1), 64 * b:64 * (b + 1)],
                              idw16[:])
        for ch in range(2):
            ek = ekpool.tile([128, HW], F32, name=f"ek{ch}", tag=f"ek{ch}")
            nc.gpsimd.ap_gather(
                out_ap=ek[:], in_ap=cbT_pl[ch][:],
                idxs_ap=idxw_all[:, b * (HW // 16):(b + 1) * (HW // 16)],
                channels=128, num_elems=K, d=1, num_idxs=HW,
            )
            cs = slice(ch * 128, (ch + 1) * 128)
            nc.scalar.dma_start(ek_ap[b, cs, :], ek[:])
            # e_k_st = fl(z + fl(e_k - z))  (reference fp32 rounding)
            d1 = ekpool.tile([128, HW], F32, name=f"st{ch}", tag=f"st{ch}")
            nc.vector.tensor_tensor(d1[:], ek[:], z_tiles[b][ch][:],
                                    op=mybir.AluOpType.subtract)
            nc.vector.tensor_tensor(d1[:], d1[:], z_tiles[b][ch][:],
                                    op=mybir.AluOpType.add)
            nc.sync.dma_start(st_ap[b, cs, :], d1[:])

    emit_batch(0)
    emit_batch(1)
    emit_gather(0)
    emit_batch(2)
    emit_gather(1)
    emit_batch(3)
    emit_gather(2)
    emit_gather(3)


def _build():
    nc = bacc.Bacc(
        "TRN2", target_bir_lowering=False, debug=False,
        enable_asserts=False, num_devices=NCORES,
    )
    z_d = nc.dram_tensor("z_e", [BPC, C, H, W], F32, kind="ExternalInput")
    cb_d = nc.dram_tensor("codebook", [K, C], F32, kind="ExternalInput")
    ek_d = nc.dram_tensor("e_k", [BPC, C, H, W], F32, kind="ExternalOutput")
    st_d = nc.dram_tensor("e_k_st", [BPC, C, H, W], F32, kind="ExternalOutput")
    ids_d = nc.dram_tensor("ids", [BPC, H, W], I32, kind="ExternalOutput")
    idsc_d = nc.dram_tensor("idscratch", [BPC, HW], I16, kind="Internal")

    z_ap = z_d.ap().rearrange("b c h w -> b c (h w)")
    ek_ap = ek_d.ap().rearrange("b c h w -> b c (h w)")
    st_ap = st_d.ap().rearrange("b c h w -> b c (h w)")
    ids_ap = ids_d.ap().rearrange("b h w -> b (h w)")

    with tile.TileContext(nc) as tc:
        with ExitStack() as ctx:
            _emit(ctx, tc, z_ap, cb_d.ap(), ek_ap, st_ap, ids_ap, idsc_d.ap())
    nc.compile()
    return nc


_NC_CACHE = []


def kernel(z_e: np.ndarray, codebook: np.ndarray):
    if not _NC_CACHE:
        _NC_CACHE.append(_build())
    nc = _NC_CACHE[0]

    z_e = np.ascontiguousarray(z_e, dtype=np.float32)
    codebook = np.ascontiguousarray(codebook, dtype=np.float32)
    in_maps = [
        {"z_e": z_e[i * BPC:(i + 1) * BPC], "codebook": codebook}
        for i in range(NCORES)
    ]
    try:
        res = run_bass_kernel_spmd(nc, in_maps, list(range(NCORES)))
    except Exception:
        # transient device state (e.g. a previous crashed run) — retry once
        res = run_bass_kernel_spmd(nc, in_maps, list(range(NCORES)))
    outs = res.results
    e_k = np.concatenate([r["e_k"] for r in outs], axis=0)
    e_k_st = np.concatenate([r["e_k_st"] for r in outs], axis=0)
    ids = np.concatenate([r["ids"] for r in outs], axis=0).astype(np.int32)
    return e_k, e_k_st, ids
